# revision 13
# baseline (speedup 1.0000x reference)
"""Trainium2 Bass kernel for nn_AdaptiveBoundaryRefinement_45861660787095.

Self-contained: takes FULL inputs (B=16,M=128,T=12000), shards batch across 8
NeuronCores (2 samples/core), runs a Bass/Tile kernel per core, gathers.

Algorithm notes (math identical to the validated baseline):
- The reference's batch-global early-stop is a mathematical no-op; the 5
  refinement iterations collapse to a closed form computed with predicated
  copies.
- Layout: per SAMPLE, time is chunked into 47 chunks of 256; chunk rows live
  on partitions 0..46, 256 steps on the free dim.  All per-sample state is
  [47, *] starting at partition 0 (satisfies the {0,32,64,96} start rule).
- v2 schedule: PSUM accumulators (S/Q/D) are split per sample so sample 0's
  entire tail chain + temporal stats overlap sample 1's mel stream; outputs
  are stored per sample.  Constants (one-hot WZ, masks) are built with
  memsets + tiny edge DMAs instead of bulk DMA loads; small input DMAs are
  spread over the tensor/gpsimd rings so the sync ring only carries mel.
- A dummy Sqrt activation is issued first so the single ACT table load
  (sqrt_and_others: sqrt+square+abs+copy) happens during the DMA lead-in;
  the cos chain uses reciprocal_approx_fast + Sqrt (no table switch).
- Temporal stats per sample: W row-sums reduce to [47,2], a ones-matmul
  collapses partitions, the scalar chain computes thresholds, and a 1-row
  ones-matmul broadcasts them back to 47 partitions (all inside one spare
  PSUM bank; 3+1 banks per sample, bufs=2 => exactly 8 banks).
"""

import sys

import numpy as np

_TRN_REPO = "/opt/trn_rl_repo"
if _TRN_REPO not in sys.path:
    sys.path.insert(0, _TRN_REPO)

import concourse.bass as bass
import concourse.bass_isa as bass_isa
import concourse.bacc as bacc
import concourse.mybir as mybir
import concourse.tile as tile
from concourse.bass_utils import run_bass_kernel_spmd

F32 = mybir.dt.float32
F32R = mybir.dt.float32r
BF16 = mybir.dt.bfloat16
ALU = mybir.AluOpType
ACTF = mybir.ActivationFunctionType
AX = mybir.AxisListType

B, M, T = 16, 128, 12000
NCORES = 8
BPC = B // NCORES            # samples per core = 2
CH = 256                     # chunk width
NCH = (T + CH - 1) // CH     # 47 chunks per sample
EPS = 1e-8
GRAD_THRESH = 0.15
LASTW = T - CH * (NCH - 1)   # 224 real cols in the last chunk
NGS = [1, 2, 4, 8, 8, 8, 8, 8]  # chunks per mel tile (ramped for fast start)
SMSC = 0.2 / 128.0


def _softmax_f32(x):
    x = np.asarray(x, np.float32)
    m = np.max(x).astype(np.float32)
    e = np.exp((x - m).astype(np.float32)).astype(np.float32)
    return (e / e.sum(dtype=np.float32).astype(np.float32)).astype(np.float32)


def build_nc(w0, w1, w2):
    nc = bacc.Bacc("TRN2", target_bir_lowering=False, debug=False)
    mel = nc.dram_tensor("mel_features", [BPC, M, T], F32R, kind="ExternalInput")
    spec = nc.dram_tensor("spectral_features", [BPC, T], F32, kind="ExternalInput")
    init = nc.dram_tensor("initial_boundaries", [BPC, T], F32, kind="ExternalInput")
    out = nc.dram_tensor("out", [BPC, T], F32, kind="ExternalOutput")

    # tiny inline const: cols 0..33 zeros, col 34 = 0.25 (edge-mask source)
    zq_np = np.zeros((1, 40), np.float32)
    zq_np[0, 34] = 0.25
    zq_d = nc.inline_tensor(zq_np, name="zq")

    th2 = float(np.float32(GRAD_THRESH) * np.float32(GRAD_THRESH))
    C1 = float(SMSC * SMSC / (T - 1))
    C2 = float(SMSC * SMSC / (float(T) * (T - 1)))

    with tile.TileContext(nc) as tc:
        with (
            tc.tile_pool(name="mel", bufs=4) as pmel,
            tc.tile_pool(name="sq", bufs=2) as psq,
            tc.tile_pool(name="cross", bufs=2) as pcross,
            tc.tile_pool(name="stat", bufs=1) as pstat,
            tc.tile_pool(name="ps", bufs=2, space="PSUM") as pps,
        ):
            # ---------------- constants (no bulk DMAs) ----------------
            # WZ(b): zeros with a ones-column at col 128; slice
            # [:, 128-r : 175-r] routes a ones-reduction into out row r (47p).
            WZ = pstat.tile([128, 257], F32R, tag="WZ")
            nc.vector.memset(WZ.bitcast(F32), 0.0)
            nc.vector.memset(WZ[:, 128:129].bitcast(F32), 1.0)
            WZb = pstat.tile([128, 257], BF16, tag="WZb")
            nc.vector.tensor_copy(out=WZb, in_=WZ)

            # dummy Sqrt first => single ACT table load (sqrt_and_others)
            dummy = pstat.tile([1, 2], F32, tag="dummy")
            nc.vector.memset(dummy, 1.0)
            nc.scalar.activation(out=dummy, in_=dummy, func=ACTF.Sqrt)

            # masks [47, *] shared by both samples (edge rows 0 and 46)
            validC = pstat.tile([47, 260], F32, tag="validC")
            nc.vector.memset(validC, 1.0)
            validD = pstat.tile([47, 256], F32, tag="validD")
            nc.vector.memset(validD, 0.1)
            scalemask = pstat.tile([47, 256], F32, tag="scalemask")
            nc.vector.memset(scalemask, 0.2)
            # edge fixes via tiny DMAs (DMA is exempt from the partition rule)
            nc.gpsimd.dma_start(out=validC[0:1, 0:2], in_=zq_d[0:1, 0:2])
            nc.gpsimd.dma_start(out=validC[46:47, 226:260], in_=zq_d[0:1, 0:34])
            nc.gpsimd.dma_start(out=validD[0:1, 0:1], in_=zq_d[0:1, 0:1])
            nc.gpsimd.dma_start(out=validD[46:47, 223:256], in_=zq_d[0:1, 0:33])
            nc.gpsimd.dma_start(out=scalemask[0:1, 1:2], in_=zq_d[0:1, 34:35])
            nc.gpsimd.dma_start(out=scalemask[46:47, 222:223], in_=zq_d[0:1, 34:35])
            # weighted masks (pool)
            CV = pstat.tile([47, 260], F32, tag="CV")
            nc.vector.tensor_scalar_mul(out=CV, in0=validC, scalar1=float(w0))
            w1vC = pstat.tile([47, 260], F32, tag="w1vC")
            nc.vector.tensor_scalar_mul(out=w1vC, in0=validC, scalar1=float(w1))

            # persistent first/last mel tiles (per sample), pads zeroed once
            W_FIRST = NGS[0] * CH + 6
            W_LAST = NGS[-1] * CH + 6
            T0_LAST = (NCH - NGS[-1]) * CH
            LASTREAL = T - (T0_LAST - 3)
            melt_firsts, melt_lasts = [], []
            for bb in range(BPC):
                mf = pstat.tile([128, W_FIRST], F32R, tag=f"mf{bb}")
                nc.gpsimd.memset(mf[:, 0:3].bitcast(F32), 0.0)
                melt_firsts.append(mf)
                ml = pstat.tile([128, W_LAST], F32R, tag=f"ml{bb}")
                nc.gpsimd.memset(ml[:, LASTREAL:W_LAST].bitcast(F32), 0.0)
                melt_lasts.append(ml)

            # ---------------- small per-sample inputs ----------------
            # specH[s]: spec at t = 256p - 3 + h, h in [0, 261)
            specHs, rs = [], []
            for s in range(BPC):
                sh = pstat.tile([47, 261], F32, tag=f"specH{s}")
                nc.vector.memset(sh, 0.0)
                specHs.append(sh)
                rr = pstat.tile([47, 256], F32, tag=f"r{s}")
                nc.vector.memset(rr, 0.0)
                rs.append(rr)

            def _dma_overlap(eng, dst, src_1d, row_lo, row_hi, col_off, width, t_base):
                ap = bass.AP(
                    tensor=src_1d.tensor,
                    offset=src_1d.offset + t_base,
                    ap=[[256, row_hi - row_lo], [1, width]],
                )
                eng.dma_start(
                    out=dst[row_lo:row_hi, col_off : col_off + width], in_=ap
                )

            for s in range(BPC):
                eng = nc.gpsimd
                sp = spec[s]
                _dma_overlap(eng, specHs[s], sp, 0, 1, 3, 258, 0)
                # spec[0] into the t=-1 slot so d=0 -> spec_sim(t=0)=1
                _dma_overlap(eng, specHs[s], sp, 0, 1, 2, 1, 0)
                _dma_overlap(eng, specHs[s], sp, 1, 46, 0, 261, 256 - 3)
                _dma_overlap(eng, specHs[s], sp, 46, 47, 0, 227, 256 * 46 - 3)
                ini = init[s]
                _dma_overlap(eng, rs[s], ini, 0, 46, 0, 256, 0)
                _dma_overlap(eng, rs[s], ini, 46, 47, 0, LASTW, 256 * 46)

            # ---------------- per-sample PSUM + streaming ----------------
            psS_l, psQ_l, psD_l = [None, None], [None, None], [None, None]
            seen = [[0, 0, 0], [0, 0, 0]]

            def emit_tile(s, j, g0, ng):
                t0 = g0 * CH
                wmel = ng * CH + 6
                if j == 0:
                    melt = melt_firsts[s]
                elif j == len(NGS) - 1:
                    melt = melt_lasts[s]
                else:
                    melt = pmel.tile([128, wmel], F32R, tag="melt")
                lo_pad = 3 if j == 0 else 0
                src_lo = t0 - 3 + lo_pad
                src_hi = min(T, t0 + ng * CH + 3)
                w_real = src_hi - src_lo
                nc.sync.dma_start(
                    out=melt[:, lo_pad : lo_pad + w_real],
                    in_=mel[s, :, src_lo:src_hi],
                )
                sq = psq.tile([128, wmel], BF16, tag="sq")
                wcr = ng * CH + 4
                cross = pcross.tile([128, wcr], BF16, tag="cross")
                if ng >= 4:
                    half = (ng // 2) * CH + 6  # covers chunk windows 0..ng/2-1
                    nc.scalar.activation(
                        out=sq[:, 0:half], in_=melt[:, 0:half], func=ACTF.Square
                    )
                    nc.vector.tensor_tensor(
                        out=cross[:, 0:half],
                        in0=melt[:, 0:half],
                        in1=melt[:, 1 : half + 1],
                        op=ALU.mult,
                    )
                    h0 = (ng // 2) * CH
                    nc.scalar.activation(
                        out=sq[:, h0:wmel], in_=melt[:, h0:wmel], func=ACTF.Square
                    )
                    nc.vector.tensor_tensor(
                        out=cross[:, h0:wcr],
                        in0=melt[:, h0:wcr],
                        in1=melt[:, h0 + 1 : wcr + 1],
                        op=ALU.mult,
                    )
                else:
                    nc.scalar.activation(out=sq, in_=melt, func=ACTF.Square)
                    nc.vector.tensor_tensor(
                        out=cross, in0=melt[:, 0:wcr], in1=melt[:, 1 : wcr + 1],
                        op=ALU.mult,
                    )
                psS_, psQ_, psD_ = psS_l[s], psQ_l[s], psD_l[s]
                for k in range(ng):
                    row = g0 + k
                    lhsT = WZ[:, 128 - row : 175 - row]
                    lhsTb = WZb[:, 128 - row : 175 - row]
                    c0 = k * CH
                    for ti, (pst, w_, lh, rhs) in enumerate(
                        (
                            (psS_, 260, lhsT, melt[:, c0 + 1 : c0 + 261]),
                            (psQ_, 262, lhsTb, sq[:, c0 : c0 + 262]),
                            (psD_, 260, lhsTb, cross[:, c0 : c0 + 260]),
                        )
                    ):
                        seen[s][ti] += 1
                        nc.tensor.matmul(
                            out=pst[0:47, 0:w_],
                            lhsT=lh,
                            rhs=rhs,
                            start=(seen[s][ti] == 1),
                            stop=(seen[s][ti] == NCH),
                        )

            def emit_spec_chain(s):
                # spec-sim chain, all on pool except the DVE reciprocal
                specH = specHs[s]
                d = pstat.tile([47, 260], F32, tag=f"d{s}")
                nc.gpsimd.tensor_tensor(
                    out=d, in0=specH[:, 1:261], in1=specH[:, 0:260],
                    op=ALU.subtract,
                )
                ab = pstat.tile([47, 260], F32, tag=f"ab{s}")
                nc.scalar.activation(out=ab, in_=d, func=ACTF.Abs)
                a1 = pstat.tile([47, 260], F32, tag=f"a1{s}")
                nc.vector.tensor_scalar_add(out=a1, in0=ab, scalar1=1.0)
                srec = pstat.tile([47, 260], F32, tag=f"srec{s}")
                nc.vector.reciprocal_approx_fast(out=srec, in_=a1)
                t1sv = pstat.tile([47, 260], F32, tag=f"t1sv{s}")
                nc.gpsimd.tensor_tensor(out=t1sv, in0=srec, in1=w1vC, op=ALU.mult)
                return t1sv

            def emit_precompute(s):
                # refinement quantities that depend only on the initial r
                r = rs[s]
                rU = pstat.tile([47, 256], F32, tag=f"rU{s}")
                nc.vector.tensor_scalar(
                    out=rU, in0=r, scalar1=0.5, scalar2=1.0,
                    op0=ALU.add, op1=ALU.min,
                )
                rDA = pstat.tile([47, 256], F32, tag=f"rDA{s}")
                nc.vector.tensor_scalar(
                    out=rDA, in0=r, scalar1=0.5, scalar2=0.0,
                    op0=ALU.subtract, op1=ALU.max,
                )
                # k = ceil(10*r-5) in (0..5]: count r > 0.5+0.1j (cmp0 = g05)
                cmps = []
                for jth in range(5):
                    c = pstat.tile([47, 256], F32, tag=f"cmp{jth}{s}")
                    nc.vector.tensor_scalar(
                        out=c, in0=r, scalar1=float(0.5 + 0.1 * jth),
                        scalar2=None, op0=ALU.is_gt,
                    )
                    cmps.append(c)
                g05 = cmps[0]
                ka = pstat.tile([47, 256], F32, tag=f"ka{s}")
                nc.gpsimd.tensor_tensor(out=ka, in0=cmps[0], in1=cmps[1], op=ALU.add)
                kb = pstat.tile([47, 256], F32, tag=f"kb{s}")
                nc.gpsimd.tensor_tensor(out=kb, in0=cmps[2], in1=cmps[3], op=ALU.add)
                nc.gpsimd.tensor_tensor(out=ka, in0=ka, in1=kb, op=ALU.add)
                nc.gpsimd.tensor_tensor(out=ka, in0=ka, in1=cmps[4], op=ALU.add)
                rD0 = pstat.tile([47, 256], F32, tag=f"rD0{s}")
                nc.vector.scalar_tensor_tensor(
                    out=rD0, in0=ka, scalar=-0.1, in1=r, op0=ALU.mult, op1=ALU.add
                )
                return g05, rU, rDA, rD0

            chains = [None, None]  # (t1sv, g05, rU, rDA, rD0) per sample

            def emit_tail(s):
                t1sv, g05, rU, rDA, rD0 = chains[s]
                psS_, psQ_, psD_ = psS_l[s], psQ_l[s], psD_l[s]
                r = rs[s]

                # --- temporal stats -> thresholds (mostly off-DVE) ---
                W = pstat.tile([47, 256], F32, tag=f"W{s}")
                psS_ap = psS_[0:47, 0:1]
                win5 = bass.AP(
                    tensor=psS_ap.tensor, offset=psS_ap.offset,
                    ap=[[512, 47], [1, 256], [1, 5]],
                )
                nc.vector.tensor_reduce(out=W, in_=win5, axis=AX.X, op=ALU.add)
                Wsq = pstat.tile([47, 256], F32, tag=f"Wsq{s}")
                nc.scalar.activation(out=Wsq, in_=W, func=ACTF.Square)
                Wstat = pstat.tile([47, 2], F32, tag=f"Wstat{s}")
                nc.vector.tensor_reduce(
                    out=Wstat[:, 0:1], in_=W, axis=AX.X, op=ALU.add
                )
                nc.vector.tensor_reduce(
                    out=Wstat[:, 1:2], in_=Wsq, axis=AX.X, op=ALU.add
                )
                sums47 = pstat.tile([47, 2], F32, tag=f"sums47{s}")
                nc.gpsimd.partition_all_reduce(
                    sums47, Wstat, channels=47, reduce_op=bass_isa.ReduceOp.add
                )
                # var = b*C1 - a^2*C2 ; std = sqrt(var) (per-partition copies)
                asq = pstat.tile([47, 1], F32, tag=f"asq{s}")
                nc.vector.tensor_tensor(
                    out=asq, in0=sums47[:, 0:1], in1=sums47[:, 0:1], op=ALU.mult
                )
                bC1 = pstat.tile([47, 1], F32, tag=f"bC1{s}")
                nc.vector.tensor_scalar_mul(out=bC1, in0=sums47[:, 1:2], scalar1=C1)
                var = pstat.tile([47, 1], F32, tag=f"var{s}")
                nc.vector.scalar_tensor_tensor(
                    out=var, in0=asq, scalar=-C2, in1=bC1, op0=ALU.mult, op1=ALU.add
                )
                std = pstat.tile([47, 1], F32, tag=f"std{s}")
                nc.scalar.activation(out=std, in_=var, func=ACTF.Sqrt)
                thr = pstat.tile([47, 2], F32, tag=f"thr{s}")
                nc.vector.tensor_scalar(
                    out=thr[:, 0:1], in0=std, scalar1=float(w2),
                    scalar2=float(0.7 - w2), op0=ALU.mult, op1=ALU.add,
                )
                nc.vector.tensor_scalar(
                    out=thr[:, 1:2], in0=std, scalar1=float(w2),
                    scalar2=float(0.4 - w2), op0=ALU.mult, op1=ALU.add,
                )

                # --- cons chain: rsqrt(Q) per element, shifted product ---
                qrI = pstat.tile([47, 261], F32, tag=f"qrI{s}")
                nc.vector.reciprocal_approx_fast(out=qrI, in_=psQ_[0:47, 0:261])
                rrtI = pstat.tile([47, 261], F32, tag=f"rrtI{s}")
                nc.scalar.activation(out=rrtI, in_=qrI, func=ACTF.Sqrt)
                prod = pstat.tile([47, 260], F32, tag=f"prod{s}")
                nc.vector.tensor_tensor(
                    out=prod, in0=rrtI[:, 0:260], in1=rrtI[:, 1:261], op=ALU.mult
                )
                rec2 = pstat.tile([47, 260], F32, tag=f"rec2{s}")
                nc.vector.tensor_tensor(out=rec2, in0=prod, in1=CV, op=ALU.mult)
                cosm = pstat.tile([47, 260], F32, tag=f"cosm{s}")
                nc.vector.tensor_tensor(
                    out=cosm, in0=psD_[0:47, 0:260], in1=rec2, op=ALU.mult
                )
                consH = pstat.tile([47, 260], F32, tag=f"consH{s}")
                nc.vector.tensor_tensor(out=consH, in0=cosm, in1=t1sv, op=ALU.add)

                # grads branch: g on DVE, square on ACT, compare on pool
                g = pstat.tile([47, 256], F32, tag=f"g{s}")
                nc.vector.tensor_tensor(
                    out=g, in0=consH[:, 2:258], in1=consH[:, 1:257],
                    op=ALU.subtract,
                )
                gsq = pstat.tile([47, 256], F32, tag=f"gsq{s}")
                nc.scalar.activation(out=gsq, in_=g, func=ACTF.Square)
                A = pstat.tile([47, 256], F32, tag=f"A{s}")
                nc.vector.tensor_scalar(
                    out=A, in0=gsq, scalar1=th2, scalar2=None, op0=ALU.is_gt
                )
                act0 = pstat.tile([47, 256], F32, tag=f"act0{s}")
                nc.vector.tensor_tensor(out=act0, in0=g05, in1=A, op=ALU.max)
                nA = pstat.tile([47, 256], F32, tag=f"nA{s}")
                nc.vector.tensor_scalar(
                    out=nA, in0=A, scalar1=-1.0, scalar2=1.0,
                    op0=ALU.mult, op1=ALU.add,
                )

                # local means via sliding-window reduce; scalemask has /5, /4
                LS = pstat.tile([47, 256], F32, tag=f"LS{s}")
                ch_ap = consH[0:47, 0:1]
                win5c = bass.AP(
                    tensor=ch_ap.tensor, offset=ch_ap.offset,
                    ap=[[260, 47], [1, 256], [1, 5]],
                )
                nc.vector.tensor_reduce(out=LS, in_=win5c, axis=AX.X, op=ALU.add)
                local = pstat.tile([47, 256], F32, tag=f"local{s}")
                nc.vector.tensor_tensor(out=local, in0=LS, in1=scalemask, op=ALU.mult)

                u = pstat.tile([47, 256], F32, tag=f"u{s}")
                nc.vector.tensor_scalar(
                    out=u, in0=local, scalar1=thr[:, 0:1], scalar2=None,
                    op0=ALU.is_gt,
                )
                v = pstat.tile([47, 256], F32, tag=f"v{s}")
                nc.vector.tensor_scalar(
                    out=v, in0=local, scalar1=thr[:, 1:2], scalar2=None,
                    op0=ALU.is_lt,
                )
                ddir = pstat.tile([47, 256], F32, tag=f"ddir{s}")
                nc.vector.tensor_tensor(out=ddir, in0=v, in1=u, op=ALU.subtract)
                nc.vector.tensor_tensor(out=ddir, in0=ddir, in1=validD, op=ALU.mult)

                up = pstat.tile([47, 256], F32, tag=f"up{s}")
                nc.vector.tensor_scalar(
                    out=up, in0=ddir, scalar1=0.0, scalar2=None, op0=ALU.is_gt
                )
                dn = pstat.tile([47, 256], F32, tag=f"dn{s}")
                nc.vector.tensor_scalar(
                    out=dn, in0=ddir, scalar1=0.0, scalar2=None, op0=ALU.is_lt
                )
                mU = pstat.tile([47, 256], F32, tag=f"mU{s}")
                nc.vector.tensor_tensor(out=mU, in0=up, in1=act0, op=ALU.mult)
                mDA = pstat.tile([47, 256], F32, tag=f"mDA{s}")
                nc.gpsimd.tensor_tensor(out=mDA, in0=dn, in1=A, op=ALU.mult)
                mD0 = pstat.tile([47, 256], F32, tag=f"mD0{s}")
                nc.gpsimd.tensor_tensor(out=mD0, in0=dn, in1=nA, op=ALU.mult)
                nc.gpsimd.tensor_tensor(out=mD0, in0=mD0, in1=g05, op=ALU.mult)
                nc.vector.copy_predicated(
                    out=r, mask=mU.bitcast(mybir.dt.int32), data=rU
                )
                nc.vector.copy_predicated(
                    out=r, mask=mDA.bitcast(mybir.dt.int32), data=rDA
                )
                nc.vector.copy_predicated(
                    out=r, mask=mD0.bitcast(mybir.dt.int32), data=rD0
                )
                nc.vector.tensor_scalar(
                    out=r, in0=r, scalar1=0.0, scalar2=1.0, op0=ALU.max, op1=ALU.min
                )

            def alloc_psum(s):
                psS_l[s] = pps.tile([128, 512], F32, tag="psS", name=f"psS{s}")
                psQ_l[s] = pps.tile([128, 512], F32, tag="psQ", name=f"psQ{s}")
                psD_l[s] = pps.tile([128, 512], F32, tag="psD", name=f"psD{s}")

            # ---- sample 0 stream ----
            alloc_psum(0)
            g0 = 0
            for j, ng in enumerate(NGS):
                emit_tile(0, j, g0, ng)
                g0 += ng
                if j == 1:
                    t1sv0 = emit_spec_chain(0)
                if j == 2:
                    pc0 = emit_precompute(0)
                    chains[0] = (t1sv0,) + pc0
            # ---- sample 1 stream start, sample 0 tail overlapped ----
            alloc_psum(1)
            g0 = 0
            for j, ng in enumerate(NGS):
                emit_tile(1, j, g0, ng)
                g0 += ng
                if j == 1:
                    emit_tail(0)
                if j == 2:
                    t1sv1 = emit_spec_chain(1)
                if j == 3:
                    pc1 = emit_precompute(1)
                    chains[1] = (t1sv1,) + pc1
            emit_tail(1)

            # ---------------- outputs (sync ring, after all mel issues) ----
            for s in range(BPC):
                ob = out[s]
                nc.sync.dma_start(
                    out=bass.AP(
                        tensor=ob.tensor, offset=ob.offset, ap=[[256, 46], [1, 256]]
                    ),
                    in_=rs[s][0:46, :],
                )
                nc.sync.dma_start(
                    out=bass.AP(
                        tensor=ob.tensor,
                        offset=ob.offset + 256 * 46,
                        ap=[[256, 1], [1, LASTW]],
                    ),
                    in_=rs[s][46:47, 0:LASTW],
                )

    nc.compile()
    return nc


_CACHE = {}


def _get_nc(wbytes):
    if wbytes not in _CACHE:
        w = np.frombuffer(wbytes, np.float32)
        _CACHE[wbytes] = build_nc(float(w[0]), float(w[1]), float(w[2]))
    return _CACHE[wbytes]


def kernel(**inputs):
    mel = np.ascontiguousarray(np.asarray(inputs["mel_features"], np.float32))
    spec = np.ascontiguousarray(np.asarray(inputs["spectral_features"], np.float32))
    init = np.ascontiguousarray(np.asarray(inputs["initial_boundaries"], np.float32))
    sw = np.asarray(inputs["similarity_weights"], np.float32)
    w = _softmax_f32(sw)
    nc = _get_nc(w.tobytes())

    in_maps = []
    for c in range(NCORES):
        s = slice(c * BPC, (c + 1) * BPC)
        in_maps.append(
            {
                "mel_features": np.ascontiguousarray(mel[s]),
                "spectral_features": np.ascontiguousarray(spec[s]),
                "initial_boundaries": np.ascontiguousarray(init[s]),
            }
        )
    res = run_bass_kernel_spmd(nc, in_maps, core_ids=list(range(NCORES)))
    global _LAST_RESULT
    _LAST_RESULT = res
    outs = [np.asarray(res.results[c]["out"], np.float32) for c in range(NCORES)]
    return np.concatenate(outs, axis=0)


_LAST_RESULT = None


if __name__ == "__main__":
    nc = build_nc(1 / 3, 1 / 3, 1 / 3)
    ninst = sum(len(b.instructions) for b in nc.m.functions[0].blocks)
    print("built ok, instructions:", ninst)


# revision 21
# speedup vs baseline: 1.1339x; 1.1339x over previous
"""Trainium2 Bass kernel for nn_AdaptiveBoundaryRefinement_45861660787095.

Self-contained: takes FULL inputs (B=16,M=128,T=12000), shards batch across 8
NeuronCores (2 samples/core), runs a Bass/Tile kernel per core, gathers.

Schedule (v6):
- Combined chunk layout [111, *]: sample 0 rows on partitions 0..46,
  sample 1 rows on partitions 64..110 (64 is a legal compute offset);
  partitions 47..63 are junk and never read back.  All elementwise chains
  and the whole tail run as single [111, *] ops (DVE cost is free-size
  bound, so this halves tail work vs per-sample tails).
- PSUM: one bank set (S/Q/D); each sample's matmuls write its partition
  range with its own start/stop accumulation group.
- Tail closed form is pure arithmetic:
    r5 = clip(r + 0.5*(v*act0v - u*Av) - (u*nAg)*kk, 0, 1)
  kk = 0.1*ceil(10r-5) = 0.05*sum_j sign(r-0.5-0.1j) + 0.25.
- Temporal stats per sample: win5 reduce of psS -> W rows, row sums via ACT
  accum_out, gpsimd partition_all_reduce per sample range; one combined
  var/std/threshold chain.
- Constants via memsets + tiny strided edge DMAs; small input DMAs deferred
  into the stream; mel tiles ramp up for sample 0 and ramp down for
  sample 1 (fast pipeline fill and drain).
- Single ACT table load (sqrt_and_others) via a dummy Sqrt.
"""

import sys

import numpy as np

_TRN_REPO = "/opt/trn_rl_repo"
if _TRN_REPO not in sys.path:
    sys.path.insert(0, _TRN_REPO)

import concourse.bass as bass
import concourse.bass_isa as bass_isa
import concourse.bacc as bacc
import concourse.mybir as mybir
import concourse.tile as tile
from concourse.bass_utils import run_bass_kernel_spmd

F32 = mybir.dt.float32
F32R = mybir.dt.float32r
BF16 = mybir.dt.bfloat16
ALU = mybir.AluOpType
ACTF = mybir.ActivationFunctionType
AX = mybir.AxisListType

B, M, T = 16, 128, 12000
NCORES = 8
BPC = B // NCORES            # samples per core = 2
CH = 256                     # chunk width
NCH = (T + CH - 1) // CH     # 47 chunks per sample
GRAD_THRESH = 0.15
LASTW = T - CH * (NCH - 1)   # 224 real cols in the last chunk
NGS0 = [1, 2, 4, 8, 8, 8, 8, 8]  # sample 0: ramp up
NGS1 = [8, 8, 8, 8, 8, 4, 2, 1]  # sample 1: ramp down
NGS_ALL = [NGS0, NGS1]
SMSC = 0.2 / 128.0
NP = 111                     # combined partition extent (47 + 17 junk + 47)
OFF = [64, 0]                # s0 at 64 (slack moves), s1 at 0 (unshifted)


def _softmax_f32(x):
    x = np.asarray(x, np.float32)
    m = np.max(x).astype(np.float32)
    e = np.exp((x - m).astype(np.float32)).astype(np.float32)
    return (e / e.sum(dtype=np.float32).astype(np.float32)).astype(np.float32)


def build_nc(w0, w1, w2):
    nc = bacc.Bacc("TRN2", target_bir_lowering=False, debug=False)
    mel = nc.dram_tensor("mel_features", [BPC, M, T], F32R, kind="ExternalInput")
    spec = nc.dram_tensor("spectral_features", [BPC, T], F32, kind="ExternalInput")
    init = nc.dram_tensor("initial_boundaries", [BPC, T], F32, kind="ExternalInput")
    out = nc.dram_tensor("out", [BPC, T], F32, kind="ExternalOutput")

    # tiny inline const: cols 0..33 zeros, col 34 = 0.25 (edge-mask source)
    zq_np = np.zeros((1, 40), np.float32)
    zq_np[0, 34] = 0.25
    zq_d = nc.inline_tensor(zq_np, name="zq")

    th2 = float(np.float32(GRAD_THRESH) * np.float32(GRAD_THRESH))
    C1 = float(SMSC * SMSC / (T - 1))
    C2 = float(SMSC * SMSC / (float(T) * (T - 1)))

    with tile.TileContext(nc) as tc:
        with (
            tc.tile_pool(name="mel", bufs=6) as pmel,
            tc.tile_pool(name="sq", bufs=3) as psq,
            tc.tile_pool(name="cross", bufs=3) as pcross,
            tc.tile_pool(name="stat", bufs=1) as pstat,
            tc.tile_pool(name="ps", bufs=2, space="PSUM") as pps,
        ):
            # ---------------- constants (no bulk DMAs) ----------------
            WZ = pstat.tile([128, 257], F32R, tag="WZ")
            nc.vector.memset(WZ.bitcast(F32), 0.0)
            nc.vector.memset(WZ[:, 128:129].bitcast(F32), 1.0)
            WZb = pstat.tile([128, 257], BF16, tag="WZb")
            nc.vector.tensor_copy(out=WZb, in_=WZ)

            # dummy Sqrt first => single ACT table load (sqrt_and_others)
            dummy = pstat.tile([1, 2], F32, tag="dummy")
            nc.vector.memset(dummy, 1.0)
            nc.scalar.activation(out=dummy, in_=dummy, func=ACTF.Sqrt)

            # masks [NP, *]; edge rows {0,46} + {64,110} via strided DMAs
            validC = pstat.tile([NP, 260], F32, tag="validC")
            nc.vector.memset(validC, 1.0)
            valid01 = pstat.tile([NP, 256], F32, tag="valid01")
            nc.vector.memset(valid01, 1.0)
            scalemask = pstat.tile([NP, 256], F32, tag="scalemask")
            nc.vector.memset(scalemask, 0.2)
            CV = pstat.tile([NP, 260], F32, tag="CV")
            w1vC = pstat.tile([NP, 260], F32, tag="w1vC")

            def _edge(dst, pitch, row0, col, width, zcol):
                # rows {row0, row0+64}, cols [col, col+width) <- zq[zcol...]
                base = dst[row0 : row0 + 1, col : col + width]
                ap = bass.AP(
                    tensor=base.tensor,
                    offset=base.offset,
                    ap=[[64 * pitch, 2], [1, width]],
                )
                zs = zq_d[0:1, zcol : zcol + width]
                src_ap = bass.AP(
                    tensor=zs.tensor,
                    offset=zs.offset,
                    ap=[[0, 2], [1, width]],
                )
                nc.gpsimd.dma_start(out=ap, in_=src_ap)

            def emit_edges():
                _edge(validC, 260, 0, 0, 2, 0)
                _edge(validC, 260, 46, 226, 34, 0)
                _edge(valid01, 256, 0, 0, 1, 0)
                _edge(valid01, 256, 46, 223, 33, 0)
                _edge(scalemask, 256, 0, 1, 1, 34)
                _edge(scalemask, 256, 46, 222, 1, 34)
                nc.vector.tensor_scalar_mul(out=CV, in0=validC, scalar1=float(w0))
                nc.vector.tensor_scalar_mul(out=w1vC, in0=validC, scalar1=float(w1))

            # persistent first/last mel tiles (per sample), pads zeroed once
            melt_firsts, melt_lasts = [], []
            for bb in range(BPC):
                ngs = NGS_ALL[bb]
                w_first = ngs[0] * CH + 6
                w_last = ngs[-1] * CH + 6
                lastreal = T - ((NCH - ngs[-1]) * CH - 3)
                mf = pstat.tile([128, w_first], F32R, tag=f"mf{bb}")
                nc.gpsimd.memset(mf[:, 0:3].bitcast(F32), 0.0)
                melt_firsts.append(mf)
                ml = pstat.tile([128, w_last], F32R, tag=f"ml{bb}")
                nc.gpsimd.memset(ml[:, lastreal:w_last].bitcast(F32), 0.0)
                melt_lasts.append(ml)

            # ---------------- small inputs (combined tiles) ----------------
            specH = pstat.tile([NP, 261], F32, tag="specH")
            nc.vector.memset(specH, 0.0)
            rin = pstat.tile([NP, 256], F32, tag="rin")
            nc.vector.memset(rin, 0.0)
            rnew = pstat.tile([NP, 256], F32, tag="rnew")

            def _dma_overlap(dst, src_1d, row_lo, row_hi, col_off, width, t_base):
                ap = bass.AP(
                    tensor=src_1d.tensor,
                    offset=src_1d.offset + t_base,
                    ap=[[256, row_hi - row_lo], [1, width]],
                )
                nc.gpsimd.dma_start(
                    out=dst[row_lo:row_hi, col_off : col_off + width], in_=ap
                )

            def emit_small_dmas(s):
                o = OFF[s]
                sp = spec[s]
                _dma_overlap(specH, sp, o, o + 1, 3, 258, 0)
                # spec[0] into the t=-1 slot so d=0 -> spec_sim(t=0)=1
                _dma_overlap(specH, sp, o, o + 1, 2, 1, 0)
                _dma_overlap(specH, sp, o + 1, o + 46, 0, 261, 256 - 3)
                _dma_overlap(specH, sp, o + 46, o + 47, 0, 227, 256 * 46 - 3)
                ini = init[s]
                _dma_overlap(rin, ini, o, o + 46, 0, 256, 0)
                _dma_overlap(rin, ini, o + 46, o + 47, 0, LASTW, 256 * 46)

            sgbias = pstat.tile([NP, 5], F32, tag="sgbias")
            for jth in range(5):
                nc.vector.memset(
                    sgbias[:, jth : jth + 1], float(-(0.5 + 0.1 * jth))
                )

            # ---------------- PSUM (per-sample bank sets) ----------------
            psS_l, psQ_l, psD_l = [None, None], [None, None], [None, None]
            seen = [[0, 0, 0], [0, 0, 0]]

            def alloc_psum(s):
                psS_l[s] = pps.tile([128, 512], F32, tag="psS", name=f"psS{s}")
                psQ_l[s] = pps.tile([128, 512], F32, tag="psQ", name=f"psQ{s}")
                psD_l[s] = pps.tile([128, 512], F32, tag="psD", name=f"psD{s}")

            def emit_tile(s, j, g0, ng):
                o = OFF[s]
                t0 = g0 * CH
                wmel = ng * CH + 6
                if j == 0:
                    melt = melt_firsts[s]
                elif j == len(NGS_ALL[s]) - 1:
                    melt = melt_lasts[s]
                else:
                    melt = pmel.tile([128, wmel], F32R, tag="melt", name="melt")
                lo_pad = 3 if j == 0 else 0
                src_lo = t0 - 3 + lo_pad
                src_hi = min(T, t0 + ng * CH + 3)
                w_real = src_hi - src_lo
                nc.sync.dma_start(
                    out=melt[:, lo_pad : lo_pad + w_real],
                    in_=mel[s, :, src_lo:src_hi],
                )
                sq = psq.tile([128, wmel], BF16, tag="sq", name="sq")
                wcr = ng * CH + 4
                cross = pcross.tile([128, wcr], BF16, tag="cross", name="cross")
                if ng >= 4:
                    half = (ng // 2) * CH + 6
                    nc.scalar.activation(
                        out=sq[:, 0:half], in_=melt[:, 0:half], func=ACTF.Square
                    )
                    nc.vector.tensor_tensor(
                        out=cross[:, 0:half],
                        in0=melt[:, 0:half],
                        in1=melt[:, 1 : half + 1],
                        op=ALU.mult,
                    )
                    h0 = (ng // 2) * CH
                    nc.scalar.activation(
                        out=sq[:, h0:wmel], in_=melt[:, h0:wmel], func=ACTF.Square
                    )
                    nc.vector.tensor_tensor(
                        out=cross[:, h0:wcr],
                        in0=melt[:, h0:wcr],
                        in1=melt[:, h0 + 1 : wcr + 1],
                        op=ALU.mult,
                    )
                else:
                    nc.scalar.activation(out=sq, in_=melt, func=ACTF.Square)
                    nc.vector.tensor_tensor(
                        out=cross, in0=melt[:, 0:wcr], in1=melt[:, 1 : wcr + 1],
                        op=ALU.mult,
                    )
                psS_, psQ_, psD_ = psS_l[s], psQ_l[s], psD_l[s]
                for k in range(ng):
                    row = g0 + k
                    lhsT = WZ[:, 128 - row : 175 - row]
                    lhsTb = WZb[:, 128 - row : 175 - row]
                    c0 = k * CH
                    for ti, (pst, w_, lh, rhs) in enumerate(
                        (
                            (psS_, 260, lhsT, melt[:, c0 + 1 : c0 + 261]),
                            (psQ_, 262, lhsTb, sq[:, c0 : c0 + 262]),
                            (psD_, 260, lhsTb, cross[:, c0 : c0 + 260]),
                        )
                    ):
                        seen[s][ti] += 1
                        nc.tensor.matmul(
                            out=pst[0:47, 0:w_],
                            lhsT=lh,
                            rhs=rhs,
                            start=(seen[s][ti] == 1),
                            stop=(seen[s][ti] == NCH),
                        )

            def emit_spec_chain():
                d = pstat.tile([NP, 260], F32, tag="d")
                nc.gpsimd.tensor_tensor(
                    out=d, in0=specH[:, 1:261], in1=specH[:, 0:260],
                    op=ALU.subtract,
                )
                ab = pstat.tile([NP, 260], F32, tag="ab")
                nc.scalar.activation(out=ab, in_=d, func=ACTF.Abs)
                a1 = pstat.tile([NP, 260], F32, tag="a1")
                nc.vector.tensor_scalar_add(out=a1, in0=ab, scalar1=1.0)
                srec = pstat.tile([NP, 260], F32, tag="srec")
                nc.vector.reciprocal_approx_fast(out=srec, in_=a1)
                t1sv = pstat.tile([NP, 260], F32, tag="t1sv")
                nc.gpsimd.tensor_tensor(out=t1sv, in0=srec, in1=w1vC, op=ALU.mult)
                return t1sv

            def emit_precompute():
                sgs = []
                for jth in range(5):
                    c = pstat.tile([NP, 256], F32, tag=f"sg{jth}", name=f"sg{jth}")
                    nc.scalar.activation(
                        out=c, in_=rin, func=ACTF.Sign, scale=1.0,
                        bias=sgbias[:, jth : jth + 1],
                    )
                    sgs.append(c)
                g05 = pstat.tile([NP, 256], F32, tag="g05")
                nc.scalar.activation(out=g05, in_=sgs[0], func=ACTF.Relu)
                ka = pstat.tile([NP, 256], F32, tag="ka")
                kb = pstat.tile([NP, 256], F32, tag="kb")
                nc.gpsimd.tensor_tensor(out=kb, in0=sgs[2], in1=sgs[3], op=ALU.add)
                nc.gpsimd.tensor_tensor(out=ka, in0=sgs[0], in1=sgs[1], op=ALU.add)
                nc.gpsimd.tensor_tensor(out=ka, in0=ka, in1=kb, op=ALU.add)
                nc.gpsimd.tensor_tensor(out=ka, in0=ka, in1=sgs[4], op=ALU.add)
                # kk = 0.1*k = 0.05*(2k-5) + 0.25
                kkb = pstat.tile([NP, 256], F32, tag="kkb")
                nc.vector.tensor_scalar(
                    out=kkb, in0=ka, scalar1=0.05, scalar2=0.25,
                    op0=ALU.mult, op1=ALU.add,
                )
                return g05, kkb

            QcC = pstat.tile([NP, 261], F32, tag="QcC")
            DcC = pstat.tile([NP, 260], F32, tag="DcC")

            def emit_cons_pre(s):
                # move psQ/psD rows into the combined SBUF tiles; sample 0
                # goes through partition-0 copies + early DMAs (slack),
                # sample 1 copies straight to partitions 0:47 (no shift).
                o = OFF[s]
                if o == 0:
                    nc.scalar.activation(
                        out=QcC[0:47, :], in_=psQ_l[s][0:47, 0:261],
                        func=ACTF.Copy,
                    )
                    nc.scalar.activation(
                        out=DcC[0:47, :], in_=psD_l[s][0:47, 0:260],
                        func=ACTF.Copy,
                    )
                else:
                    q0c = pstat.tile([47, 261], F32, tag="q0c")
                    nc.scalar.activation(
                        out=q0c, in_=psQ_l[s][0:47, 0:261], func=ACTF.Copy
                    )
                    nc.gpsimd.dma_start(out=QcC[o : o + 47, :], in_=q0c)
                    d0c = pstat.tile([47, 260], F32, tag="d0c")
                    nc.scalar.activation(
                        out=d0c, in_=psD_l[s][0:47, 0:260], func=ACTF.Copy
                    )
                    nc.gpsimd.dma_start(out=DcC[o : o + 47, :], in_=d0c)

            thrC = pstat.tile([NP, 2], F32, tag="thrC")
            thrs = [None, None]

            def emit_temporal(s):
                """Per-sample temporal chain at partition 0; thr -> DMA."""
                o = OFF[s]
                W = pstat.tile([47, 256], F32, tag=f"W{s}", name=f"W{s}")
                ps_ap = psS_l[s][0:1, 0:1]
                win5 = bass.AP(
                    tensor=ps_ap.tensor, offset=ps_ap.offset,
                    ap=[[512, 47], [1, 256], [1, 5]],
                )
                nc.vector.tensor_reduce(out=W, in_=win5, axis=AX.X, op=ALU.add)
                Wsq = pstat.tile([47, 256], F32, tag=f"Wsq{s}", name=f"Wsq{s}")
                Wstat = pstat.tile([47, 2], F32, tag=f"Wstat{s}", name=f"Wstat{s}")
                nc.scalar.activation(
                    out=Wsq, in_=W, func=ACTF.Copy, accum_out=Wstat[:, 0:1]
                )
                nc.scalar.activation(
                    out=Wsq, in_=W, func=ACTF.Square, accum_out=Wstat[:, 1:2]
                )
                sums = pstat.tile([47, 2], F32, tag=f"sums{s}", name=f"sums{s}")
                nc.gpsimd.partition_all_reduce(
                    sums, Wstat, channels=47, reduce_op=bass_isa.ReduceOp.add
                )
                asq = pstat.tile([47, 1], F32, tag=f"asq{s}", name=f"asq{s}")
                nc.vector.tensor_tensor(
                    out=asq, in0=sums[:, 0:1], in1=sums[:, 0:1], op=ALU.mult
                )
                bC1 = pstat.tile([47, 1], F32, tag=f"bC1{s}", name=f"bC1{s}")
                nc.vector.tensor_scalar_mul(out=bC1, in0=sums[:, 1:2], scalar1=C1)
                var = pstat.tile([47, 1], F32, tag=f"var{s}", name=f"var{s}")
                nc.vector.scalar_tensor_tensor(
                    out=var, in0=asq, scalar=-C2, in1=bC1, op0=ALU.mult, op1=ALU.add
                )
                std = pstat.tile([47, 1], F32, tag=f"std{s}", name=f"std{s}")
                nc.scalar.activation(out=std, in_=var, func=ACTF.Sqrt)
                thr = pstat.tile([47, 2], F32, tag=f"thr{s}", name=f"thr{s}")
                nc.vector.tensor_scalar(
                    out=thr[:, 0:1], in0=std, scalar1=float(w2),
                    scalar2=float(0.7 - w2), op0=ALU.mult, op1=ALU.add,
                )
                nc.vector.tensor_scalar(
                    out=thr[:, 1:2], in0=std, scalar1=float(w2),
                    scalar2=float(0.4 - w2), op0=ALU.mult, op1=ALU.add,
                )
                if o != 0:
                    nc.gpsimd.dma_start(out=thrC[o : o + 47, :], in_=thr)
                thrs[s] = thr

            def emit_tail(t1sv, g05, kkb):
                qrI = pstat.tile([NP, 261], F32, tag="qrI")
                nc.vector.reciprocal_approx_fast(out=qrI, in_=QcC)
                rrtI = pstat.tile([NP, 261], F32, tag="rrtI")
                nc.scalar.activation(out=rrtI, in_=qrI, func=ACTF.Sqrt)
                prod = pstat.tile([NP, 260], F32, tag="prod")
                nc.vector.tensor_tensor(
                    out=prod, in0=rrtI[:, 0:260], in1=rrtI[:, 1:261], op=ALU.mult
                )
                rec2 = pstat.tile([NP, 260], F32, tag="rec2")
                nc.vector.tensor_tensor(out=rec2, in0=prod, in1=CV, op=ALU.mult)
                cosm = pstat.tile([NP, 260], F32, tag="cosm")
                nc.vector.tensor_tensor(out=cosm, in0=DcC, in1=rec2, op=ALU.mult)
                consH = pstat.tile([NP, 260], F32, tag="consH")
                nc.vector.tensor_tensor(out=consH, in0=cosm, in1=t1sv, op=ALU.add)
                # grads branch (pool handles the valid01 products in parallel)
                g = pstat.tile([NP, 256], F32, tag="g")
                nc.vector.tensor_tensor(
                    out=g, in0=consH[:, 2:258], in1=consH[:, 1:257],
                    op=ALU.subtract,
                )
                gsq = pstat.tile([NP, 256], F32, tag="gsq")
                nc.scalar.activation(out=gsq, in_=g, func=ACTF.Square)
                A = pstat.tile([NP, 256], F32, tag="A")
                nc.vector.tensor_scalar(
                    out=A, in0=gsq, scalar1=th2, scalar2=None, op0=ALU.is_gt
                )
                act0 = pstat.tile([NP, 256], F32, tag="act0")
                nc.vector.tensor_tensor(out=act0, in0=g05, in1=A, op=ALU.max)
                act0v = pstat.tile([NP, 256], F32, tag="act0v")
                nc.gpsimd.tensor_tensor(out=act0v, in0=act0, in1=valid01, op=ALU.mult)
                Av = pstat.tile([NP, 256], F32, tag="Av")
                nc.gpsimd.tensor_tensor(out=Av, in0=A, in1=valid01, op=ALU.mult)
                nAg = pstat.tile([NP, 256], F32, tag="nAg")
                nc.gpsimd.tensor_tensor(out=nAg, in0=act0v, in1=Av, op=ALU.subtract)
                # local means
                LS = pstat.tile([NP, 256], F32, tag="LS")
                ch_ap = consH[0:1, 0:1]
                win5c = bass.AP(
                    tensor=ch_ap.tensor, offset=ch_ap.offset,
                    ap=[[260, NP], [1, 256], [1, 5]],
                )
                nc.vector.tensor_reduce(out=LS, in_=win5c, axis=AX.X, op=ALU.add)
                local = pstat.tile([NP, 256], F32, tag="local")
                nc.vector.tensor_tensor(out=local, in0=LS, in1=scalemask, op=ALU.mult)
                u = pstat.tile([NP, 256], F32, tag="u")
                v = pstat.tile([NP, 256], F32, tag="v")
                # s1 rows (0:47): direct per-sample thr; s0 rows (64:111):
                # thrC filled early by DMA -- all operands offset-matched
                nc.vector.tensor_scalar(
                    out=u[0:47, :], in0=local[0:47, :],
                    scalar1=thrs[1][:, 0:1], scalar2=None, op0=ALU.is_gt,
                )
                nc.vector.tensor_scalar(
                    out=v[0:47, :], in0=local[0:47, :],
                    scalar1=thrs[1][:, 1:2], scalar2=None, op0=ALU.is_lt,
                )
                nc.vector.tensor_scalar(
                    out=u[64:111, :], in0=local[64:111, :],
                    scalar1=thrC[64:111, 0:1], scalar2=None, op0=ALU.is_gt,
                )
                nc.vector.tensor_scalar(
                    out=v[64:111, :], in0=local[64:111, :],
                    scalar1=thrC[64:111, 1:2], scalar2=None, op0=ALU.is_lt,
                )
                # r5 = clip(r + 0.5*(v*act0v - u*Av) - (u*nAg)*kk)
                mU = pstat.tile([NP, 256], F32, tag="mU")
                nc.vector.tensor_tensor(out=mU, in0=v, in1=act0v, op=ALU.mult)
                mDA = pstat.tile([NP, 256], F32, tag="mDA")
                nc.vector.tensor_tensor(out=mDA, in0=u, in1=Av, op=ALU.mult)
                mD0 = pstat.tile([NP, 256], F32, tag="mD0")
                nc.vector.tensor_tensor(out=mD0, in0=u, in1=nAg, op=ALU.mult)
                e = pstat.tile([NP, 256], F32, tag="e")
                nc.vector.tensor_tensor(out=e, in0=mU, in1=mDA, op=ALU.subtract)
                nc.vector.scalar_tensor_tensor(
                    out=rnew, in0=e, scalar=0.5, in1=rin, op0=ALU.mult, op1=ALU.add
                )
                f = pstat.tile([NP, 256], F32, tag="f")
                nc.vector.tensor_tensor(out=f, in0=mD0, in1=kkb, op=ALU.mult)
                nc.vector.tensor_tensor(out=rnew, in0=rnew, in1=f, op=ALU.subtract)
                nc.vector.tensor_scalar(
                    out=rnew, in0=rnew, scalar1=0.0, scalar2=1.0,
                    op0=ALU.max, op1=ALU.min,
                )

            # ---- sample 0 stream ----
            alloc_psum(0)
            g0 = 0
            t1sv = g05 = kkb = None
            for j, ng in enumerate(NGS0):
                emit_tile(0, j, g0, ng)
                g0 += ng
                if j == 0:
                    emit_small_dmas(0)
                if j == 1:
                    emit_edges()
                if j == 2:
                    emit_small_dmas(1)
            # ---- sample 1 stream ----
            alloc_psum(1)
            g0 = 0
            for j, ng in enumerate(NGS1):
                emit_tile(1, j, g0, ng)
                g0 += ng
                if j == 0:
                    t1sv = emit_spec_chain()
                if j == 1:
                    g05, kkb = emit_precompute()
                if j == 2:
                    emit_temporal(0)
                if j == 3:
                    emit_cons_pre(0)
            emit_temporal(1)
            emit_cons_pre(1)
            emit_tail(t1sv, g05, kkb)

            # ---------------- outputs (sync ring, after all mel issues) ----
            for s in range(BPC):
                o = OFF[s]
                ob = out[s]
                nc.sync.dma_start(
                    out=bass.AP(
                        tensor=ob.tensor, offset=ob.offset, ap=[[256, 46], [1, 256]]
                    ),
                    in_=rnew[o : o + 46, :],
                )
                nc.sync.dma_start(
                    out=bass.AP(
                        tensor=ob.tensor,
                        offset=ob.offset + 256 * 46,
                        ap=[[256, 1], [1, LASTW]],
                    ),
                    in_=rnew[o + 46 : o + 47, 0:LASTW],
                )

    nc.compile()
    return nc


_CACHE = {}


def _get_nc(wbytes):
    if wbytes not in _CACHE:
        w = np.frombuffer(wbytes, np.float32)
        _CACHE[wbytes] = build_nc(float(w[0]), float(w[1]), float(w[2]))
    return _CACHE[wbytes]


def kernel(**inputs):
    mel = np.ascontiguousarray(np.asarray(inputs["mel_features"], np.float32))
    spec = np.ascontiguousarray(np.asarray(inputs["spectral_features"], np.float32))
    init = np.ascontiguousarray(np.asarray(inputs["initial_boundaries"], np.float32))
    sw = np.asarray(inputs["similarity_weights"], np.float32)
    w = _softmax_f32(sw)
    nc = _get_nc(w.tobytes())

    in_maps = []
    for c in range(NCORES):
        s = slice(c * BPC, (c + 1) * BPC)
        in_maps.append(
            {
                "mel_features": np.ascontiguousarray(mel[s]),
                "spectral_features": np.ascontiguousarray(spec[s]),
                "initial_boundaries": np.ascontiguousarray(init[s]),
            }
        )
    res = run_bass_kernel_spmd(nc, in_maps, core_ids=list(range(NCORES)))
    global _LAST_RESULT
    _LAST_RESULT = res
    outs = [np.asarray(res.results[c]["out"], np.float32) for c in range(NCORES)]
    return np.concatenate(outs, axis=0)


_LAST_RESULT = None


if __name__ == "__main__":
    nc = build_nc(1 / 3, 1 / 3, 1 / 3)
    ninst = sum(len(b.instructions) for b in nc.m.functions[0].blocks)
    print("built ok, instructions:", ninst)


# revision 22
# speedup vs baseline: 1.1614x; 1.0242x over previous
"""Trainium2 Bass kernel for nn_AdaptiveBoundaryRefinement_45861660787095.

Self-contained: takes FULL inputs (B=16,M=128,T=12000), shards batch across 8
NeuronCores (2 samples/core), runs a Bass/Tile kernel per core, gathers.

Schedule (v6):
- Combined chunk layout [111, *]: sample 0 rows on partitions 0..46,
  sample 1 rows on partitions 64..110 (64 is a legal compute offset);
  partitions 47..63 are junk and never read back.  All elementwise chains
  and the whole tail run as single [111, *] ops (DVE cost is free-size
  bound, so this halves tail work vs per-sample tails).
- PSUM: one bank set (S/Q/D); each sample's matmuls write its partition
  range with its own start/stop accumulation group.
- Tail closed form is pure arithmetic:
    r5 = clip(r + 0.5*(v*act0v - u*Av) - (u*nAg)*kk, 0, 1)
  kk = 0.1*ceil(10r-5) = 0.05*sum_j sign(r-0.5-0.1j) + 0.25.
- Temporal stats per sample: win5 reduce of psS -> W rows, row sums via ACT
  accum_out, gpsimd partition_all_reduce per sample range; one combined
  var/std/threshold chain.
- Constants via memsets + tiny strided edge DMAs; small input DMAs deferred
  into the stream; mel tiles ramp up for sample 0 and ramp down for
  sample 1 (fast pipeline fill and drain).
- Single ACT table load (sqrt_and_others) via a dummy Sqrt.
"""

import sys

import numpy as np

_TRN_REPO = "/opt/trn_rl_repo"
if _TRN_REPO not in sys.path:
    sys.path.insert(0, _TRN_REPO)

import concourse.bass as bass
import concourse.bass_isa as bass_isa
import concourse.bacc as bacc
import concourse.mybir as mybir
import concourse.tile as tile
from concourse.bass_utils import run_bass_kernel_spmd

F32 = mybir.dt.float32
F32R = mybir.dt.float32r
BF16 = mybir.dt.bfloat16
ALU = mybir.AluOpType
ACTF = mybir.ActivationFunctionType
AX = mybir.AxisListType

B, M, T = 16, 128, 12000
NCORES = 8
BPC = B // NCORES            # samples per core = 2
CH = 256                     # chunk width
NCH = (T + CH - 1) // CH     # 47 chunks per sample
GRAD_THRESH = 0.15
LASTW = T - CH * (NCH - 1)   # 224 real cols in the last chunk
NGS0 = [1, 2, 4, 8, 8, 8, 8, 8]  # sample 0: ramp up
NGS1 = [8, 8, 8, 8, 8, 4, 2, 1]  # sample 1: ramp down
NGS_ALL = [NGS0, NGS1]
SMSC = 0.2 / 128.0
NP = 111                     # combined partition extent (47 + 17 junk + 47)
OFF = [64, 0]                # s0 at 64 (slack moves), s1 at 0 (unshifted)


def _softmax_f32(x):
    x = np.asarray(x, np.float32)
    m = np.max(x).astype(np.float32)
    e = np.exp((x - m).astype(np.float32)).astype(np.float32)
    return (e / e.sum(dtype=np.float32).astype(np.float32)).astype(np.float32)


def build_nc(w0, w1, w2):
    nc = bacc.Bacc("TRN2", target_bir_lowering=False, debug=False)
    mel = nc.dram_tensor("mel_features", [BPC, M, T], F32R, kind="ExternalInput")
    spec = nc.dram_tensor("spectral_features", [BPC, T], F32, kind="ExternalInput")
    init = nc.dram_tensor("initial_boundaries", [BPC, T], F32, kind="ExternalInput")
    out = nc.dram_tensor("out", [BPC, T], F32, kind="ExternalOutput")

    # tiny inline const: cols 0..33 zeros, col 34 = 0.25 (edge-mask source)
    zq_np = np.zeros((1, 40), np.float32)
    zq_np[0, 34] = 0.25
    zq_d = nc.inline_tensor(zq_np, name="zq")

    th2 = float(np.float32(GRAD_THRESH) * np.float32(GRAD_THRESH))
    C1 = float(SMSC * SMSC / (T - 1))
    C2 = float(SMSC * SMSC / (float(T) * (T - 1)))

    with tile.TileContext(nc) as tc:
        with (
            tc.tile_pool(name="mel", bufs=6) as pmel,
            tc.tile_pool(name="sq", bufs=3) as psq,
            tc.tile_pool(name="cross", bufs=3) as pcross,
            tc.tile_pool(name="stat", bufs=1) as pstat,
            tc.tile_pool(name="ps", bufs=2, space="PSUM") as pps,
        ):
            # ---------------- constants (no bulk DMAs) ----------------
            WZ = pstat.tile([128, 257], F32R, tag="WZ")
            nc.vector.memset(WZ.bitcast(F32), 0.0)
            nc.vector.memset(WZ[:, 128:129].bitcast(F32), 1.0)
            WZb = pstat.tile([128, 257], BF16, tag="WZb")
            nc.vector.tensor_copy(out=WZb, in_=WZ)

            # dummy Sqrt first => single ACT table load (sqrt_and_others)
            dummy = pstat.tile([1, 2], F32, tag="dummy")
            nc.vector.memset(dummy, 1.0)
            nc.scalar.activation(out=dummy, in_=dummy, func=ACTF.Sqrt)

            # masks [NP, *]; edge rows {0,46} + {64,110} via strided DMAs
            validC = pstat.tile([NP, 260], F32, tag="validC")
            nc.vector.memset(validC, 1.0)
            valid01 = pstat.tile([NP, 256], F32, tag="valid01")
            nc.vector.memset(valid01, 1.0)
            scalemask = pstat.tile([NP, 256], F32, tag="scalemask")
            nc.vector.memset(scalemask, 0.2)
            CV = pstat.tile([NP, 260], F32, tag="CV")
            w1vC = pstat.tile([NP, 260], F32, tag="w1vC")

            def _edge(dst, pitch, row0, col, width, zcol):
                # rows {row0, row0+64}, cols [col, col+width) <- zq[zcol...]
                base = dst[row0 : row0 + 1, col : col + width]
                ap = bass.AP(
                    tensor=base.tensor,
                    offset=base.offset,
                    ap=[[64 * pitch, 2], [1, width]],
                )
                zs = zq_d[0:1, zcol : zcol + width]
                src_ap = bass.AP(
                    tensor=zs.tensor,
                    offset=zs.offset,
                    ap=[[0, 2], [1, width]],
                )
                nc.gpsimd.dma_start(out=ap, in_=src_ap)

            def emit_edges():
                _edge(validC, 260, 0, 0, 2, 0)
                _edge(validC, 260, 46, 226, 34, 0)
                _edge(valid01, 256, 0, 0, 1, 0)
                _edge(valid01, 256, 46, 223, 33, 0)
                _edge(scalemask, 256, 0, 1, 1, 34)
                _edge(scalemask, 256, 46, 222, 1, 34)
                nc.vector.tensor_scalar_mul(out=CV, in0=validC, scalar1=float(w0))
                nc.vector.tensor_scalar_mul(out=w1vC, in0=validC, scalar1=float(w1))

            # persistent first/last mel tiles (per sample), pads zeroed once
            melt_firsts, melt_lasts = [], []
            for bb in range(BPC):
                ngs = NGS_ALL[bb]
                w_first = ngs[0] * CH + 6
                w_last = ngs[-1] * CH + 6
                lastreal = T - ((NCH - ngs[-1]) * CH - 3)
                mf = pstat.tile([128, w_first], F32R, tag=f"mf{bb}")
                nc.gpsimd.memset(mf[:, 0:3].bitcast(F32), 0.0)
                melt_firsts.append(mf)
                ml = pstat.tile([128, w_last], F32R, tag=f"ml{bb}")
                nc.gpsimd.memset(ml[:, lastreal:w_last].bitcast(F32), 0.0)
                melt_lasts.append(ml)

            # ---------------- small inputs (combined tiles) ----------------
            specH = pstat.tile([NP, 261], F32, tag="specH")
            nc.vector.memset(specH, 0.0)
            rin = pstat.tile([NP, 256], F32, tag="rin")
            nc.vector.memset(rin, 0.0)
            rnew = pstat.tile([NP, 256], F32, tag="rnew")

            def _dma_overlap(dst, src_1d, row_lo, row_hi, col_off, width, t_base):
                ap = bass.AP(
                    tensor=src_1d.tensor,
                    offset=src_1d.offset + t_base,
                    ap=[[256, row_hi - row_lo], [1, width]],
                )
                nc.gpsimd.dma_start(
                    out=dst[row_lo:row_hi, col_off : col_off + width], in_=ap
                )

            def emit_small_dmas(s):
                o = OFF[s]
                sp = spec[s]
                _dma_overlap(specH, sp, o, o + 1, 3, 258, 0)
                # spec[0] into the t=-1 slot so d=0 -> spec_sim(t=0)=1
                _dma_overlap(specH, sp, o, o + 1, 2, 1, 0)
                _dma_overlap(specH, sp, o + 1, o + 46, 0, 261, 256 - 3)
                _dma_overlap(specH, sp, o + 46, o + 47, 0, 227, 256 * 46 - 3)
                ini = init[s]
                _dma_overlap(rin, ini, o, o + 46, 0, 256, 0)
                _dma_overlap(rin, ini, o + 46, o + 47, 0, LASTW, 256 * 46)

            sgbias = pstat.tile([NP, 5], F32, tag="sgbias")
            for jth in range(5):
                nc.vector.memset(
                    sgbias[:, jth : jth + 1], float(-(0.5 + 0.1 * jth))
                )

            # ---------------- PSUM (per-sample bank sets) ----------------
            psS_l, psQ_l, psD_l = [None, None], [None, None], [None, None]
            seen = [[0, 0, 0], [0, 0, 0]]

            def alloc_psum(s):
                psS_l[s] = pps.tile([128, 512], F32, tag="psS", name=f"psS{s}")
                psQ_l[s] = pps.tile([128, 512], F32, tag="psQ", name=f"psQ{s}")
                psD_l[s] = pps.tile([128, 512], F32, tag="psD", name=f"psD{s}")

            def emit_tile(s, j, g0, ng):
                o = OFF[s]
                t0 = g0 * CH
                wmel = ng * CH + 6
                if j == 0:
                    melt = melt_firsts[s]
                elif j == len(NGS_ALL[s]) - 1:
                    melt = melt_lasts[s]
                else:
                    melt = pmel.tile([128, wmel], F32R, tag="melt", name="melt")
                lo_pad = 3 if j == 0 else 0
                src_lo = t0 - 3 + lo_pad
                src_hi = min(T, t0 + ng * CH + 3)
                w_real = src_hi - src_lo
                nc.sync.dma_start(
                    out=melt[:, lo_pad : lo_pad + w_real],
                    in_=mel[s, :, src_lo:src_hi],
                )
                sq = psq.tile([128, wmel], BF16, tag="sq", name="sq")
                wcr = ng * CH + 4
                cross = pcross.tile([128, wcr], BF16, tag="cross", name="cross")
                if ng >= 4:
                    half = (ng // 2) * CH + 6
                    nc.scalar.activation(
                        out=sq[:, 0:half], in_=melt[:, 0:half], func=ACTF.Square
                    )
                    nc.vector.tensor_tensor(
                        out=cross[:, 0:half],
                        in0=melt[:, 0:half],
                        in1=melt[:, 1 : half + 1],
                        op=ALU.mult,
                    )
                    h0 = (ng // 2) * CH
                    nc.scalar.activation(
                        out=sq[:, h0:wmel], in_=melt[:, h0:wmel], func=ACTF.Square
                    )
                    nc.vector.tensor_tensor(
                        out=cross[:, h0:wcr],
                        in0=melt[:, h0:wcr],
                        in1=melt[:, h0 + 1 : wcr + 1],
                        op=ALU.mult,
                    )
                else:
                    nc.scalar.activation(out=sq, in_=melt, func=ACTF.Square)
                    nc.vector.tensor_tensor(
                        out=cross, in0=melt[:, 0:wcr], in1=melt[:, 1 : wcr + 1],
                        op=ALU.mult,
                    )
                psS_, psQ_, psD_ = psS_l[s], psQ_l[s], psD_l[s]
                for k in range(ng):
                    row = g0 + k
                    lhsT = WZ[:, 128 - row : 175 - row]
                    lhsTb = WZb[:, 128 - row : 175 - row]
                    c0 = k * CH
                    for ti, (pst, w_, lh, rhs) in enumerate(
                        (
                            (psS_, 260, lhsT, melt[:, c0 + 1 : c0 + 261]),
                            (psQ_, 262, lhsTb, sq[:, c0 : c0 + 262]),
                            (psD_, 260, lhsTb, cross[:, c0 : c0 + 260]),
                        )
                    ):
                        seen[s][ti] += 1
                        nc.tensor.matmul(
                            out=pst[0:47, 0:w_],
                            lhsT=lh,
                            rhs=rhs,
                            start=(seen[s][ti] == 1),
                            stop=(seen[s][ti] == NCH),
                        )

            def emit_spec_chain():
                d = pstat.tile([NP, 260], F32, tag="d")
                nc.gpsimd.tensor_tensor(
                    out=d, in0=specH[:, 1:261], in1=specH[:, 0:260],
                    op=ALU.subtract,
                )
                ab = pstat.tile([NP, 260], F32, tag="ab")
                nc.scalar.activation(out=ab, in_=d, func=ACTF.Abs)
                a1 = pstat.tile([NP, 260], F32, tag="a1")
                nc.vector.tensor_scalar_add(out=a1, in0=ab, scalar1=1.0)
                srec = pstat.tile([NP, 260], F32, tag="srec")
                nc.vector.reciprocal_approx_fast(out=srec, in_=a1)
                t1sv = pstat.tile([NP, 260], F32, tag="t1sv")
                nc.gpsimd.tensor_tensor(out=t1sv, in0=srec, in1=w1vC, op=ALU.mult)
                return t1sv

            def emit_precompute():
                sgs = []
                for jth in range(5):
                    c = pstat.tile([NP, 256], F32, tag=f"sg{jth}", name=f"sg{jth}")
                    nc.scalar.activation(
                        out=c, in_=rin, func=ACTF.Sign, scale=1.0,
                        bias=sgbias[:, jth : jth + 1],
                    )
                    sgs.append(c)
                g05 = pstat.tile([NP, 256], F32, tag="g05")
                nc.scalar.activation(out=g05, in_=sgs[0], func=ACTF.Relu)
                ka = pstat.tile([NP, 256], F32, tag="ka")
                kb = pstat.tile([NP, 256], F32, tag="kb")
                nc.gpsimd.tensor_tensor(out=kb, in0=sgs[2], in1=sgs[3], op=ALU.add)
                nc.gpsimd.tensor_tensor(out=ka, in0=sgs[0], in1=sgs[1], op=ALU.add)
                nc.gpsimd.tensor_tensor(out=ka, in0=ka, in1=kb, op=ALU.add)
                nc.gpsimd.tensor_tensor(out=ka, in0=ka, in1=sgs[4], op=ALU.add)
                # kk = 0.1*k = 0.05*(2k-5) + 0.25
                kkb = pstat.tile([NP, 256], F32, tag="kkb")
                nc.vector.tensor_scalar(
                    out=kkb, in0=ka, scalar1=0.05, scalar2=0.25,
                    op0=ALU.mult, op1=ALU.add,
                )
                return g05, kkb

            QcC = pstat.tile([NP, 261], F32, tag="QcC")
            DcC = pstat.tile([NP, 260], F32, tag="DcC")

            def emit_cons_pre(s):
                # move psQ/psD rows into the combined SBUF tiles; sample 0
                # goes through partition-0 copies + early DMAs (slack),
                # sample 1 copies straight to partitions 0:47 (no shift).
                o = OFF[s]
                if o == 0:
                    nc.scalar.activation(
                        out=QcC[0:47, :], in_=psQ_l[s][0:47, 0:261],
                        func=ACTF.Copy,
                    )
                    nc.scalar.activation(
                        out=DcC[0:47, :], in_=psD_l[s][0:47, 0:260],
                        func=ACTF.Copy,
                    )
                else:
                    q0c = pstat.tile([47, 261], F32, tag="q0c")
                    nc.scalar.activation(
                        out=q0c, in_=psQ_l[s][0:47, 0:261], func=ACTF.Copy
                    )
                    nc.gpsimd.dma_start(out=QcC[o : o + 47, :], in_=q0c)
                    d0c = pstat.tile([47, 260], F32, tag="d0c")
                    nc.scalar.activation(
                        out=d0c, in_=psD_l[s][0:47, 0:260], func=ACTF.Copy
                    )
                    nc.gpsimd.dma_start(out=DcC[o : o + 47, :], in_=d0c)

            thrC = pstat.tile([NP, 2], F32, tag="thrC")
            thrs = [None, None]

            def emit_temporal(s):
                """Per-sample temporal chain at partition 0; thr -> DMA."""
                o = OFF[s]
                W = pstat.tile([47, 256], F32, tag=f"W{s}", name=f"W{s}")
                ps_ap = psS_l[s][0:1, 0:1]
                win5 = bass.AP(
                    tensor=ps_ap.tensor, offset=ps_ap.offset,
                    ap=[[512, 47], [1, 256], [1, 5]],
                )
                nc.vector.tensor_reduce(out=W, in_=win5, axis=AX.X, op=ALU.add)
                Wsq = pstat.tile([47, 256], F32, tag=f"Wsq{s}", name=f"Wsq{s}")
                Wstat = pstat.tile([47, 2], F32, tag=f"Wstat{s}", name=f"Wstat{s}")
                nc.scalar.activation(
                    out=Wsq, in_=W, func=ACTF.Copy, accum_out=Wstat[:, 0:1]
                )
                nc.scalar.activation(
                    out=Wsq, in_=W, func=ACTF.Square, accum_out=Wstat[:, 1:2]
                )
                sums = pstat.tile([47, 2], F32, tag=f"sums{s}", name=f"sums{s}")
                nc.gpsimd.partition_all_reduce(
                    sums, Wstat, channels=47, reduce_op=bass_isa.ReduceOp.add
                )
                asq = pstat.tile([47, 1], F32, tag=f"asq{s}", name=f"asq{s}")
                nc.vector.tensor_tensor(
                    out=asq, in0=sums[:, 0:1], in1=sums[:, 0:1], op=ALU.mult
                )
                bC1 = pstat.tile([47, 1], F32, tag=f"bC1{s}", name=f"bC1{s}")
                nc.vector.tensor_scalar_mul(out=bC1, in0=sums[:, 1:2], scalar1=C1)
                var = pstat.tile([47, 1], F32, tag=f"var{s}", name=f"var{s}")
                nc.vector.scalar_tensor_tensor(
                    out=var, in0=asq, scalar=-C2, in1=bC1, op0=ALU.mult, op1=ALU.add
                )
                std = pstat.tile([47, 1], F32, tag=f"std{s}", name=f"std{s}")
                nc.scalar.activation(out=std, in_=var, func=ACTF.Sqrt)
                thr = pstat.tile([47, 2], F32, tag=f"thr{s}", name=f"thr{s}")
                nc.vector.tensor_scalar(
                    out=thr[:, 0:1], in0=std, scalar1=float(w2),
                    scalar2=float(0.7 - w2), op0=ALU.mult, op1=ALU.add,
                )
                nc.vector.tensor_scalar(
                    out=thr[:, 1:2], in0=std, scalar1=float(w2),
                    scalar2=float(0.4 - w2), op0=ALU.mult, op1=ALU.add,
                )
                if o != 0:
                    nc.gpsimd.dma_start(out=thrC[o : o + 47, :], in_=thr)
                thrs[s] = thr

            def emit_tail(t1sv, g05, kkb):
                qrI = pstat.tile([NP, 261], F32, tag="qrI")
                nc.vector.reciprocal_approx_fast(out=qrI, in_=QcC)
                rrtI = pstat.tile([NP, 261], F32, tag="rrtI")
                nc.scalar.activation(out=rrtI, in_=qrI, func=ACTF.Sqrt)
                prod = pstat.tile([NP, 260], F32, tag="prod")
                nc.vector.tensor_tensor(
                    out=prod, in0=rrtI[:, 0:260], in1=rrtI[:, 1:261], op=ALU.mult
                )
                rec2 = pstat.tile([NP, 260], F32, tag="rec2")
                nc.vector.tensor_tensor(out=rec2, in0=prod, in1=CV, op=ALU.mult)
                cosm = pstat.tile([NP, 260], F32, tag="cosm")
                nc.vector.tensor_tensor(out=cosm, in0=DcC, in1=rec2, op=ALU.mult)
                consH = pstat.tile([NP, 260], F32, tag="consH")
                nc.vector.tensor_tensor(out=consH, in0=cosm, in1=t1sv, op=ALU.add)
                # grads branch (pool handles the valid01 products in parallel)
                g = pstat.tile([NP, 256], F32, tag="g")
                nc.vector.tensor_tensor(
                    out=g, in0=consH[:, 2:258], in1=consH[:, 1:257],
                    op=ALU.subtract,
                )
                gsq = pstat.tile([NP, 256], F32, tag="gsq")
                nc.scalar.activation(out=gsq, in_=g, func=ACTF.Square)
                A = pstat.tile([NP, 256], F32, tag="A")
                nc.vector.tensor_scalar(
                    out=A, in0=gsq, scalar1=th2, scalar2=None, op0=ALU.is_gt
                )
                act0 = pstat.tile([NP, 256], F32, tag="act0")
                nc.vector.tensor_tensor(out=act0, in0=g05, in1=A, op=ALU.max)
                act0v = pstat.tile([NP, 256], F32, tag="act0v")
                nc.vector.tensor_tensor(out=act0v, in0=act0, in1=valid01, op=ALU.mult)
                Av = pstat.tile([NP, 256], F32, tag="Av")
                nc.vector.tensor_tensor(out=Av, in0=A, in1=valid01, op=ALU.mult)
                nAg = pstat.tile([NP, 256], F32, tag="nAg")
                nc.vector.tensor_tensor(out=nAg, in0=act0v, in1=Av, op=ALU.subtract)
                # local means
                LS = pstat.tile([NP, 256], F32, tag="LS")
                ch_ap = consH[0:1, 0:1]
                win5c = bass.AP(
                    tensor=ch_ap.tensor, offset=ch_ap.offset,
                    ap=[[260, NP], [1, 256], [1, 5]],
                )
                nc.vector.tensor_reduce(out=LS, in_=win5c, axis=AX.X, op=ALU.add)
                local = pstat.tile([NP, 256], F32, tag="local")
                nc.vector.tensor_tensor(out=local, in0=LS, in1=scalemask, op=ALU.mult)
                u = pstat.tile([NP, 256], F32, tag="u")
                v = pstat.tile([NP, 256], F32, tag="v")
                # s1 rows (0:47): direct per-sample thr; s0 rows (64:111):
                # thrC filled early by DMA -- all operands offset-matched
                nc.vector.tensor_scalar(
                    out=u[0:47, :], in0=local[0:47, :],
                    scalar1=thrs[1][:, 0:1], scalar2=None, op0=ALU.is_gt,
                )
                nc.vector.tensor_scalar(
                    out=v[0:47, :], in0=local[0:47, :],
                    scalar1=thrs[1][:, 1:2], scalar2=None, op0=ALU.is_lt,
                )
                nc.vector.tensor_scalar(
                    out=u[64:111, :], in0=local[64:111, :],
                    scalar1=thrC[64:111, 0:1], scalar2=None, op0=ALU.is_gt,
                )
                nc.vector.tensor_scalar(
                    out=v[64:111, :], in0=local[64:111, :],
                    scalar1=thrC[64:111, 1:2], scalar2=None, op0=ALU.is_lt,
                )
                # r5 = clip(r + 0.5*(v*act0v - u*Av) - (u*nAg)*kk)
                mU = pstat.tile([NP, 256], F32, tag="mU")
                nc.vector.tensor_tensor(out=mU, in0=v, in1=act0v, op=ALU.mult)
                mDA = pstat.tile([NP, 256], F32, tag="mDA")
                nc.vector.tensor_tensor(out=mDA, in0=u, in1=Av, op=ALU.mult)
                mD0 = pstat.tile([NP, 256], F32, tag="mD0")
                nc.vector.tensor_tensor(out=mD0, in0=u, in1=nAg, op=ALU.mult)
                e = pstat.tile([NP, 256], F32, tag="e")
                nc.vector.tensor_tensor(out=e, in0=mU, in1=mDA, op=ALU.subtract)
                nc.vector.scalar_tensor_tensor(
                    out=rnew, in0=e, scalar=0.5, in1=rin, op0=ALU.mult, op1=ALU.add
                )
                f = pstat.tile([NP, 256], F32, tag="f")
                nc.vector.tensor_tensor(out=f, in0=mD0, in1=kkb, op=ALU.mult)
                nc.vector.tensor_tensor(out=rnew, in0=rnew, in1=f, op=ALU.subtract)
                nc.vector.tensor_scalar(
                    out=rnew, in0=rnew, scalar1=0.0, scalar2=1.0,
                    op0=ALU.max, op1=ALU.min,
                )

            # ---- sample 0 stream ----
            alloc_psum(0)
            g0 = 0
            t1sv = g05 = kkb = None
            for j, ng in enumerate(NGS0):
                emit_tile(0, j, g0, ng)
                g0 += ng
                if j == 0:
                    emit_small_dmas(0)
                if j == 1:
                    emit_edges()
                if j == 2:
                    emit_small_dmas(1)
                if j == 4:
                    t1sv = emit_spec_chain()
                if j == 5:
                    g05, kkb = emit_precompute()
            # ---- sample 1 stream ----
            alloc_psum(1)
            g0 = 0
            for j, ng in enumerate(NGS1):
                emit_tile(1, j, g0, ng)
                g0 += ng
                if j == 0:
                    emit_temporal(0)
                if j == 1:
                    emit_cons_pre(0)
            emit_temporal(1)
            emit_cons_pre(1)
            emit_tail(t1sv, g05, kkb)

            # ---------------- outputs (sync + scalar rings) ----
            for s in range(BPC):
                o = OFF[s]
                ob = out[s]
                eng = nc.sync if s == 0 else nc.scalar
                eng.dma_start(
                    out=bass.AP(
                        tensor=ob.tensor, offset=ob.offset, ap=[[256, 46], [1, 256]]
                    ),
                    in_=rnew[o : o + 46, :],
                )
                eng.dma_start(
                    out=bass.AP(
                        tensor=ob.tensor,
                        offset=ob.offset + 256 * 46,
                        ap=[[256, 1], [1, LASTW]],
                    ),
                    in_=rnew[o + 46 : o + 47, 0:LASTW],
                )

    nc.compile()
    return nc


_CACHE = {}


def _get_nc(wbytes):
    if wbytes not in _CACHE:
        w = np.frombuffer(wbytes, np.float32)
        _CACHE[wbytes] = build_nc(float(w[0]), float(w[1]), float(w[2]))
    return _CACHE[wbytes]


def kernel(**inputs):
    mel = np.ascontiguousarray(np.asarray(inputs["mel_features"], np.float32))
    spec = np.ascontiguousarray(np.asarray(inputs["spectral_features"], np.float32))
    init = np.ascontiguousarray(np.asarray(inputs["initial_boundaries"], np.float32))
    sw = np.asarray(inputs["similarity_weights"], np.float32)
    w = _softmax_f32(sw)
    nc = _get_nc(w.tobytes())

    in_maps = []
    for c in range(NCORES):
        s = slice(c * BPC, (c + 1) * BPC)
        in_maps.append(
            {
                "mel_features": np.ascontiguousarray(mel[s]),
                "spectral_features": np.ascontiguousarray(spec[s]),
                "initial_boundaries": np.ascontiguousarray(init[s]),
            }
        )
    res = run_bass_kernel_spmd(nc, in_maps, core_ids=list(range(NCORES)))
    global _LAST_RESULT
    _LAST_RESULT = res
    outs = [np.asarray(res.results[c]["out"], np.float32) for c in range(NCORES)]
    return np.concatenate(outs, axis=0)


_LAST_RESULT = None


if __name__ == "__main__":
    nc = build_nc(1 / 3, 1 / 3, 1 / 3)
    ninst = sum(len(b.instructions) for b in nc.m.functions[0].blocks)
    print("built ok, instructions:", ninst)
